# revision 1
# baseline (speedup 1.0000x reference)
"""Trainium2 Bass kernel for nn_Encoder (2-layer GCN encoder, graph mean readout).

Math restructuring (exact, up to float reordering):
  Layer 1 (GCNConv + ReLU):  x1 = relu(dis * S + b1),
      S[n] = sum_{e in seg(n)} y[src(e)]  (dst-segments incl. self edge),
      y[m] = dis[m] * (x[m] @ W1),  dis = (deg+1)^-1/2.
  Layer 2 + mean over nodes collapses to a per-node scalar:
      out = (1/N) * (sum_n c[n] * x1[n]) @ W2 + b2,
      c[m] = dis[m] * (sum_{e: src(e)=m} dis[dst(e)] + dis[m]).
So the device kernel only needs: one dense matmul pass producing y (fp16
rows [node, 2*H] in DRAM), one edge-gather + segmented-sum pass (SWDGE
dma_gather + one-hot matmuls into PSUM), and a tiny weighted accumulation.
The final [2,128] @ W2 happens on host (65k FLOPs of the original 13 GFLOP).

Sharding: destination nodes (and their incoming edges) are split across the
8 cores; every core computes the full y table itself (redundant compute is
cheaper than an all-gather at these sizes, and needs no collectives).
Per-core structure is IDENTICAL (SPMD: one program, data-only variation):
destination nodes are bin-packed on host into TILES tiles of 128 psum slots
with a fixed per-src-chunk chunk budget (rotating (5,4,4,4) pattern), so
every gather call / matmul schedule is a compile-time constant.
"""

import sys, os, types
sys.path.insert(0, "/opt/trn_rl_repo")

# antenv.axon_hooks shim (image's antenv stub lacks it); needed for NTFF trace.
if "antenv.axon_hooks" not in sys.modules:
    _hook = [None]
    _m = types.ModuleType("antenv.axon_hooks")
    _m.set_axon_ntff_profile_hook = lambda h: _hook.__setitem__(0, h)
    _m.get_axon_ntff_profile_hook = lambda: _hook[0]
    sys.modules["antenv.axon_hooks"] = _m
    try:
        import antenv
        antenv.axon_hooks = _m
        from trn_agent_boot.trn_boot import _ntff_profile_via_ctypes
        _m.set_axon_ntff_profile_hook(
            _ntff_profile_via_ctypes("/opt/axon/libaxon_pjrt.so"))
    except Exception:
        pass

import numpy as np
from contextlib import ExitStack
from dataclasses import dataclass

import concourse.bacc as bacc
import concourse.bass as bass
import concourse.mybir as mybir
import concourse.tile as tile
from concourse.bass_utils import run_bass_kernel_spmd
from concourse.library_config import mlp

P = 128
H = 128
F_IN = 116
FEXT = F_IN + 8          # one-hot node-type rows appended -> K=124
B = 2
YW = B * H               # 256: y row elements (both batches)


@dataclass(frozen=True)
class Cfg:
    n: int = 100000      # nodes
    ncores: int = 8
    tiles: int = 104     # dst tiles per core (128 slots each)
    chunks: int = 16     # 128-edge chunks per tile (sum over 4 src chunks)
    group: int = 2       # tiles per gather-call group (= psum tiles in flight)
                         # group*chunks/nsc*128 = call_idx must stay <= 1024:
                         # the SWDGE descriptor ring caps a single dma_gather
    nsc: int = 4         # src chunks (int16 gather index reach)

    @property
    def ndst(self):
        return self.n // self.ncores

    @property
    def srcchunk(self):
        return -(-self.n // self.nsc)

    @property
    def npad(self):       # node count padded to phase-1 block (512)
        return -(-self.n // 512) * 512

    @property
    def rot(self):        # rot[r][s]: chunks of tile (t%4==r) in src chunk s
        base, extra = divmod(self.chunks, self.nsc)
        return [[base + (1 if (s - r) % self.nsc < extra else 0)
                 for s in range(self.nsc)] for r in range(self.nsc)]

    @property
    def ngroups(self):
        return self.tiles // self.group

    @property
    def call_chunks(self):  # chunks per gather call = sum_r rot[r][s] (same all s)
        return sum(self.rot[r][0] for r in range(self.group))

    @property
    def call_idx(self):
        return self.call_chunks * P

    @property
    def ncalls(self):
        return self.ngroups * self.nsc

    @property
    def idxcols(self):
        return self.ncalls * (self.call_idx // 16)

    @property
    def nchunks_total(self):
        return self.tiles * self.chunks


CFG = Cfg()

f32 = mybir.dt.float32
f16 = mybir.dt.float16
i16 = mybir.dt.int16


def _build_program(cfg: Cfg, has_b1: bool):
    nc = bacc.Bacc("TRN2")
    xe = nc.dram_tensor("xe", [B, FEXT, cfg.npad], f32, kind="ExternalInput")
    xeo = nc.dram_tensor("xeo", [B, FEXT, cfg.tiles * P], f32,
                         kind="ExternalInput")
    w1e = nc.dram_tensor("w1e", [FEXT, H], f32, kind="ExternalInput")
    idxt = nc.dram_tensor("idxt", [P, cfg.idxcols], i16, kind="ExternalInput")
    dlt = nc.dram_tensor("dlt", [P, cfg.nchunks_total], f16, kind="ExternalInput")
    dcq = nc.dram_tensor("dcq", [P, cfg.tiles], f32, kind="ExternalInput")
    iot = nc.dram_tensor("iot", [P, P], f16, kind="ExternalInput")
    if has_b1:
        disc = nc.dram_tensor("disc", [P, cfg.tiles], f32, kind="ExternalInput")
        cct = nc.dram_tensor("cct", [P, cfg.tiles], f32, kind="ExternalInput")
        b1b = nc.dram_tensor("b1b", [P, YW], f32, kind="ExternalInput")
    y = nc.dram_tensor("y", [cfg.npad, YW], f16, kind="Internal")
    accd = nc.dram_tensor("acc", [P, YW], f32, kind="ExternalOutput")

    nblk = cfg.npad // 512
    rotpre = [[sum(cfg.rot[i][s] for i in range(r)) for s in range(cfg.nsc)]
              for r in range(cfg.group)]

    with tile.TileContext(nc) as tc:
        nc.gpsimd.load_library(mlp)
        with (
            tc.tile_pool(name="const", bufs=1) as cpool,
            tc.tile_pool(name="ph1", bufs=4) as p1pool,
            tc.tile_pool(name="ysb", bufs=3) as ypool,
            tc.tile_pool(name="gat", bufs=4) as gpool,
            tc.tile_pool(name="oh", bufs=8) as ohpool,
            tc.tile_pool(name="x1c", bufs=4) as xpool,
            tc.tile_pool(name="psy", bufs=2, space="PSUM") as psy,
            tc.tile_pool(name="psa", bufs=6, space="PSUM") as psa,
            ExitStack() as ctx,
        ):
            # constants / small preloads
            w1_sb = cpool.tile([FEXT, H], f32, tag="w1")
            nc.sync.dma_start(w1_sb[:], w1e[:])
            iota_sb = cpool.tile([P, P], f16, tag="iota")
            nc.sync.dma_start(iota_sb[:], iot[:])
            dl_sb = cpool.tile([P, cfg.nchunks_total], f16, tag="dl")
            nc.sync.dma_start(dl_sb[:], dlt[:])
            dcq_sb = cpool.tile([P, cfg.tiles], f32, tag="dcq")
            nc.sync.dma_start(dcq_sb[:], dcq[:])
            if has_b1:
                disc_sb = cpool.tile([P, cfg.tiles], f32, tag="disc")
                nc.sync.dma_start(disc_sb[:], disc[:])
                cc_sb = cpool.tile([P, cfg.tiles], f32, tag="cc")
                nc.sync.dma_start(cc_sb[:], cct[:])
                b1_sb = cpool.tile([P, YW], f32, tag="b1b")
                nc.sync.dma_start(b1_sb[:], b1b[:])
            acc_sb = cpool.tile([P, YW], f32, tag="acc")
            nc.vector.memset(acc_sb[:], 0)

            # ---- Phase 1: y[node] = dis*(x @ W1ext), fp16 rows [node, 2*H]
            for blk in range(nblk):
                n0 = blk * 512
                xts = []
                for b in range(B):
                    xt = p1pool.tile([FEXT, 512], f32, tag=f"xt{b}")
                    nc.sync.dma_start(xt[:], xe[b, :, n0:n0 + 512])
                    xts.append(xt)
                for sub in range(4):
                    ysb = ypool.tile([P, YW], f16, tag="ysb")
                    for b in range(B):
                        ps = psy.tile([P, H], f32, tag="psy")
                        nc.tensor.matmul(
                            ps[:],
                            lhsT=xts[b][:, sub * P:(sub + 1) * P],
                            rhs=w1_sb[:], start=True, stop=True)
                        nc.scalar.activation(
                            out=ysb[:, b * H:(b + 1) * H], in_=ps[:],
                            func=mybir.ActivationFunctionType.Copy)
                    r0 = n0 + sub * P
                    nc.sync.dma_start(y[r0:r0 + P, :], ysb[:])

            # ---- Phase 2: gather + segmented one-hot matmul + accumulate
            for g in range(cfg.ngroups):
                pst = [psa.tile([P, YW], f32, tag="psa", name=f"pst{g}_{i}")
                       for i in range(cfg.group)]
                # self-loop inputs: dis^2-scaled own features (see xeo build)
                xos = []
                for b in range(B):
                    xo = p1pool.tile([FEXT, cfg.group * P], f32, tag=f"xo{b}")
                    nc.sync.dma_start(
                        xo[:], xeo[b, :, g * cfg.group * P:(g + 1) * cfg.group * P])
                    xos.append(xo)
                start_mm = [None] * cfg.group
                for s in range(cfg.nsc):
                    call = g * cfg.nsc + s
                    ic0 = call * (cfg.call_idx // 16)
                    idx_sb = gpool.tile([P, cfg.call_idx // 16], i16, tag="idx")
                    nc.sync.dma_start(
                        idx_sb[:], idxt[:, ic0:ic0 + cfg.call_idx // 16])
                    gt = gpool.tile([P, cfg.call_chunks, YW], f16, tag="gt")
                    r0 = s * cfg.srcchunk
                    nc.gpsimd.dma_gather(
                        gt[:], y[r0:r0 + cfg.srcchunk, :], idx_sb[:],
                        cfg.call_idx, cfg.call_idx, YW)
                    for ti in range(cfg.group):
                        t = g * cfg.group + ti
                        k = cfg.rot[ti][s]
                        off = rotpre[ti][s]
                        for j in range(k):
                            # global chunk column for dstloc:
                            gcol = call * cfg.call_chunks + off + j
                            oh = ohpool.tile([P, P], f16, tag="oh")
                            nc.vector.tensor_tensor(
                                out=oh[:],
                                in0=dl_sb[:, gcol:gcol + 1].to_broadcast([P, P]),
                                in1=iota_sb[:],
                                op=mybir.AluOpType.is_equal)
                            # exactly one start=True matmul per psum tile (PSUM
                            # zero-regions are 2KB-wide: start marks the whole
                            # region pending-zero, so it must be unique + first)
                            is_start = (s == 0 and j == 0)
                            mm = nc.tensor.matmul(
                                pst[ti][:], lhsT=oh[:],
                                rhs=gt[:, off + j, :],
                                start=is_start,
                                stop=(s == cfg.nsc - 1 and j == k - 1))
                            if is_start:
                                start_mm[ti] = mm
                                # self-loop term: accumulate xeo @ W1ext into
                                # each batch half, after the start matmul
                                for b in range(B):
                                    sm = nc.tensor.matmul(
                                        pst[ti][:, b * H:(b + 1) * H],
                                        lhsT=xos[b][:, ti * P:(ti + 1) * P],
                                        rhs=w1_sb[:], start=False, stop=False)
                                    bass._add_dep_helper(
                                        sm.ins, start_mm[ti].ins, sync=False,
                                        reason="self-mm after psum start")
                            else:
                                bass._add_dep_helper(
                                    mm.ins, start_mm[ti].ins, sync=False,
                                    reason="accum after psum start")
                for ti in range(cfg.group):
                    t = g * cfg.group + ti
                    x1c = xpool.tile([P, YW], f32, tag="x1c")
                    if not has_b1:
                        # x1c = relu(psum * (dis*c))   (valid since c>0)
                        nc.scalar.activation(
                            out=x1c[:], in_=pst[ti][:],
                            func=mybir.ActivationFunctionType.Relu,
                            bias=0.0, scale=dcq_sb[:, t:t + 1])
                    else:
                        t1 = xpool.tile([P, YW], f32, tag="t1")
                        nc.vector.tensor_scalar(
                            out=t1[:], in0=pst[ti][:],
                            scalar1=disc_sb[:, t:t + 1], scalar2=None,
                            op0=mybir.AluOpType.mult)
                        nc.vector.tensor_tensor(
                            out=t1[:], in0=t1[:], in1=b1_sb[:],
                            op=mybir.AluOpType.add)
                        nc.scalar.activation(
                            out=t1[:], in_=t1[:],
                            func=mybir.ActivationFunctionType.Relu)
                        nc.vector.tensor_scalar(
                            out=x1c[:], in0=t1[:],
                            scalar1=cc_sb[:, t:t + 1], scalar2=None,
                            op0=mybir.AluOpType.mult)
                    nc.vector.tensor_tensor(
                        out=acc_sb[:], in0=acc_sb[:], in1=x1c[:],
                        op=mybir.AluOpType.add)

            nc.sync.dma_start(accd[:], acc_sb[:])

    nc.compile()
    return nc


_PROG_CACHE = {}


def _get_program(cfg: Cfg, has_b1: bool):
    key = (cfg, has_b1)
    if key not in _PROG_CACHE:
        _PROG_CACHE[key] = _build_program(cfg, has_b1)
    return _PROG_CACHE[key]


def _pack_core(cfg: Cfg, core, src, dst, dis_c, n_nodes):
    """Bin-pack this core's dst nodes into tiles; build gather/dstloc/dcq data.

    Returns (idx_w [128, idxcols] i16, dl_w [128, nchunks] f16,
             dcq_w [128, tiles] f32, tile_of, slot_of)."""
    n0 = core * cfg.ndst
    sel = (dst >= n0) & (dst < n0 + cfg.ndst)
    es = src[sel]
    ed = dst[sel]
    # (self edges are handled by the xeown direct matmul, not the gather)
    dl = ed - n0                       # local dst id
    sc = es // cfg.srcchunk            # src chunk of each edge

    cnt = np.bincount(dl * cfg.nsc + sc, minlength=cfg.ndst * cfg.nsc)
    cnt = cnt.reshape(cfg.ndst, cfg.nsc)

    rot = np.array(cfg.rot, dtype=np.int64)          # [4, nsc]
    caps = (rot[np.arange(cfg.tiles) % cfg.nsc] * P).copy()  # [tiles, nsc]
    for s in range(cfg.nsc):
        assert cnt[:, s].sum() <= caps[:, s].sum(), \
            f"core {core}: src chunk {s} demand exceeds capacity"

    order = np.argsort(-cnt.sum(1), kind="stable")
    slots_used = np.zeros(cfg.tiles, dtype=np.int64)
    tile_of = np.full(cfg.ndst, -1, dtype=np.int64)
    slot_of = np.full(cfg.ndst, -1, dtype=np.int64)
    for nloc in order:
        need = cnt[nloc]
        ok = (caps >= need).all(axis=1) & (slots_used < P)
        if not ok.any():
            raise RuntimeError(f"core {core}: bin packing failed for node {nloc}")
        # best fit = feasible tile with most remaining capacity (balances load;
        # with exact slot counts every tile must end up full)
        score = caps.sum(axis=1) + (P - slots_used)
        score[~ok] = -1
        t = int(np.argmax(score))
        tile_of[nloc] = t
        slot_of[nloc] = slots_used[t]
        slots_used[t] += 1
        caps[t] -= need

    # edge stream positions
    et = tile_of[dl]
    eslot = slot_of[dl]
    o = np.lexsort((sc, et))
    et_s, sc_s, slot_s, src_s = et[o], sc[o], eslot[o], es[o]
    ks = et_s * cfg.nsc + sc_s
    counts = np.bincount(ks, minlength=cfg.tiles * cfg.nsc)
    gbase = np.concatenate([[0], np.cumsum(counts)[:-1]])
    rank = np.arange(len(ks)) - gbase[ks]

    # padded stream base for (t, s)
    rotpre = np.zeros((cfg.nsc, cfg.nsc), dtype=np.int64)  # [r, s] prefix
    for r in range(cfg.nsc):
        for s in range(cfg.nsc):
            rotpre[r, s] = sum(cfg.rot[i][s] for i in range(r))
    tt = np.arange(cfg.tiles)
    callno = (tt // cfg.group)[:, None] * cfg.nsc + np.arange(cfg.nsc)[None, :]
    pbase = callno * cfg.call_idx + rotpre[tt % cfg.group] * P  # [tiles, nsc]
    assert (counts.reshape(cfg.tiles, cfg.nsc) <= rot[tt % cfg.nsc] * P).all()

    total = cfg.ncalls * cfg.call_idx
    idx_flat = np.zeros(total, dtype=np.int16)
    dl_flat = np.full(total, 255.0, dtype=np.float16)
    pos = pbase[et_s, sc_s] + rank
    idx_flat[pos] = (src_s - sc_s * cfg.srcchunk).astype(np.int16)
    dl_flat[pos] = slot_s.astype(np.float16)

    ci = cfg.call_idx
    idx_w = idx_flat.reshape(cfg.ncalls, ci // 16, 16).transpose(2, 0, 1)
    idx_w = np.tile(idx_w.reshape(16, -1), (8, 1))           # [128, idxcols]
    dl_w = dl_flat.reshape(cfg.nchunks_total, P).T.copy()    # [128, nchunks]

    dcq_w = np.zeros((P, cfg.tiles), dtype=np.float32)
    dcq_w[slot_of, tile_of] = dis_c[n0:n0 + cfg.ndst]
    return idx_w, dl_w, dcq_w, tile_of, slot_of


def _prepare(cfg: Cfg, node, node_type, edge_index, embed, W1, b1, W2, b2):
    n = cfg.n
    src = edge_index[0].astype(np.int64)
    dst = edge_index[1].astype(np.int64)
    deg = (np.bincount(dst, minlength=n) + 1).astype(np.float32)
    dis = (1.0 / np.sqrt(deg.astype(np.float64))).astype(np.float32)
    s_arr = np.bincount(src, weights=dis[dst].astype(np.float64), minlength=n)
    c = (dis.astype(np.float64) * (s_arr + dis)).astype(np.float32)
    dis_c = (dis.astype(np.float64) * c).astype(np.float32)

    T8 = (embed.astype(np.float64) @ W1[F_IN:, :].astype(np.float64))
    w1e = np.concatenate([W1[:F_IN, :], T8.astype(np.float32)], axis=0)
    w1e = np.ascontiguousarray(w1e, dtype=np.float32)

    xe = np.zeros((B, FEXT, cfg.npad), dtype=np.float32)
    xe[:, :F_IN, :n] = node.transpose(0, 2, 1) * dis[None, None, :]
    oh8 = np.zeros((8, n), dtype=np.float32)
    oh8[node_type.astype(np.int64), np.arange(n)] = dis
    xe[:, F_IN:, :n] = oh8[None]

    iota = np.tile(np.arange(P, dtype=np.float16), (P, 1))

    has_b1 = bool(np.any(b1 != 0))
    in_maps = []
    metas = []
    for core in range(cfg.ncores):
        idx_w, dl_w, dcq_w, tile_of, slot_of = _pack_core(
            cfg, core, src, dst, dis_c, n)
        # xeown: own nodes' features at (tile, slot) columns. xe already
        # carries one dis factor, so xeown @ W1ext = dis*xw = y[n], exactly the
        # self-loop row the segment sum needs (psum is scaled by dis*c later).
        n0 = core * cfg.ndst
        perm = np.full(cfg.tiles * P, -1, dtype=np.int64)
        perm[tile_of * P + slot_of] = np.arange(n0, n0 + cfg.ndst)
        used = perm >= 0
        xeo = np.zeros((B, FEXT, cfg.tiles * P), dtype=np.float32)
        xeo[:, :, used] = xe[:, :, perm[used]]
        m = {"xe": xe, "xeo": xeo, "w1e": w1e, "idxt": idx_w, "dlt": dl_w,
             "dcq": dcq_w, "iot": iota}
        if has_b1:
            disc_w = np.zeros((P, cfg.tiles), dtype=np.float32)
            cc_w = np.zeros((P, cfg.tiles), dtype=np.float32)
            n0 = core * cfg.ndst
            disc_w[slot_of, tile_of] = dis[n0:n0 + cfg.ndst]
            cc_w[slot_of, tile_of] = c[n0:n0 + cfg.ndst]
            m["disc"] = disc_w
            m["cct"] = cc_w
            m["b1b"] = np.tile(b1.astype(np.float32), (P, B))
        in_maps.append(m)
        metas.append((tile_of, slot_of))
    return in_maps, has_b1


def run(inputs, cfg: Cfg = CFG, trace: bool = False):
    node = np.asarray(inputs["node"], dtype=np.float32)
    node_type = np.asarray(inputs["node_type"])
    edge_index = np.asarray(inputs["edge_index"])
    embed = np.asarray(inputs["embed"], dtype=np.float32)
    W1 = np.asarray(inputs["W1"], dtype=np.float32)
    b1 = np.asarray(inputs["b1"], dtype=np.float32)
    W2 = np.asarray(inputs["W2"], dtype=np.float32)
    b2 = np.asarray(inputs["b2"], dtype=np.float32)

    in_maps, has_b1 = _prepare(cfg, node, node_type, edge_index,
                               embed, W1, b1, W2, b2)
    nc = _get_program(cfg, has_b1)
    res = run_bass_kernel_spmd(
        nc, in_maps, core_ids=list(range(cfg.ncores)), trace=trace,
        trace_cores=list(range(cfg.ncores)) if trace else None)

    total = np.zeros((B, H), dtype=np.float64)
    for core in range(cfg.ncores):
        acc = res.results[core]["acc"].astype(np.float64)   # [128, 2*H]
        total += acc.reshape(P, B, H).sum(axis=0)
    out = (total @ W2.astype(np.float64)) / cfg.n + b2.astype(np.float64)
    return out.astype(np.float32), res


def kernel(**inputs) -> np.ndarray:
    out, _ = run(inputs, CFG, trace=False)
    return out



# revision 8
# speedup vs baseline: 5.4313x; 5.4313x over previous
"""Trainium2 Bass kernel for nn_Encoder (2-layer GCN encoder, graph mean readout).

Math restructuring (exact, up to float reordering):
  Layer 1 (GCNConv + ReLU):  x1 = relu(dis * S + b1),
      S[n] = sum_{e in seg(n)} y[src(e)]  (dst-segments incl. self edge),
      y[m] = dis[m] * (x[m] @ W1),  dis = (deg+1)^-1/2.
  Layer 2 + mean over nodes collapses to a per-node scalar:
      out = (1/N) * (sum_n c[n] * x1[n]) @ W2 + b2,
      c[m] = dis[m] * (sum_{e: src(e)=m} dis[dst(e)] + dis[m]).
So the device kernel only needs: one dense matmul pass producing y (fp16
rows [node, 2*H] in DRAM), one edge-gather + segmented-sum pass (SWDGE
dma_gather + one-hot matmuls into PSUM), and a tiny weighted accumulation.
The final [2,128] @ W2 happens on host (65k FLOPs of the original 13 GFLOP).

Sharding: destination nodes (and their incoming edges) are split across the
8 cores; every core computes the full y table itself (redundant compute is
cheaper than an all-gather at these sizes, and needs no collectives).
Per-core structure is IDENTICAL (SPMD: one program, data-only variation):
destination nodes are bin-packed on host into TILES tiles of 128 psum slots
with a fixed per-src-chunk chunk budget (rotating (5,4,4,4) pattern), so
every gather call / matmul schedule is a compile-time constant.
"""

import sys, os, types
sys.path.insert(0, "/opt/trn_rl_repo")

# antenv.axon_hooks shim (image's antenv stub lacks it); needed for NTFF trace.
if "antenv.axon_hooks" not in sys.modules:
    _hook = [None]
    _m = types.ModuleType("antenv.axon_hooks")
    _m.set_axon_ntff_profile_hook = lambda h: _hook.__setitem__(0, h)
    _m.get_axon_ntff_profile_hook = lambda: _hook[0]
    sys.modules["antenv.axon_hooks"] = _m
    try:
        import antenv
        antenv.axon_hooks = _m
        from trn_agent_boot.trn_boot import _ntff_profile_via_ctypes
        _m.set_axon_ntff_profile_hook(
            _ntff_profile_via_ctypes("/opt/axon/libaxon_pjrt.so"))
    except Exception:
        pass

import numpy as np
from contextlib import ExitStack
from dataclasses import dataclass

import concourse.bacc as bacc
import concourse.bass as bass
import concourse.mybir as mybir
import concourse.tile as tile
from concourse.bass_utils import run_bass_kernel_spmd
from concourse.library_config import mlp

P = 128
H = 128
F_IN = 116
FEXT = F_IN + 8          # one-hot node-type rows appended -> K=124
B = 2
YW = B * H               # 256: y row elements (both batches)


@dataclass(frozen=True)
class Cfg:
    n: int = 100000      # nodes
    ncores: int = 8
    tiles: int = 104     # dst tiles per core (128 slots each)
    chunks: int = 16     # 128-edge chunks per tile (sum over 4 src chunks)
    group: int = 2       # tiles per gather-call group (= psum tiles in flight)
                         # group*chunks/nsc*128 = call_idx must stay <= 1024:
                         # the SWDGE descriptor ring caps a single dma_gather
    nsc: int = 4         # src chunks (int16 gather index reach)

    @property
    def ndst(self):
        return self.n // self.ncores

    @property
    def srcchunk(self):
        return -(-self.n // self.nsc)

    @property
    def npad(self):       # node count padded to phase-1 block (512)
        return -(-self.n // 512) * 512

    @property
    def rot(self):        # rot[r][s]: chunks of tile (t%4==r) in src chunk s
        base, extra = divmod(self.chunks, self.nsc)
        return [[base + (1 if (s - r) % self.nsc < extra else 0)
                 for s in range(self.nsc)] for r in range(self.nsc)]

    @property
    def ngroups(self):
        return self.tiles // self.group

    @property
    def call_chunks(self):  # chunks per gather call = sum_r rot[r][s] (same all s)
        return sum(self.rot[r][0] for r in range(self.group))

    @property
    def call_idx(self):
        return self.call_chunks * P

    @property
    def ncalls(self):
        return self.ngroups * self.nsc

    @property
    def idxcols(self):
        return self.ncalls * (self.call_idx // 16)

    @property
    def nchunks_total(self):
        return self.tiles * self.chunks


CFG = Cfg()

f32 = mybir.dt.float32
f16 = mybir.dt.float16
i16 = mybir.dt.int16


def _build_program(cfg: Cfg, has_b1: bool):
    nc = bacc.Bacc("TRN2", num_swdge_queues=4)
    xe = nc.dram_tensor("xe", [B, FEXT, cfg.npad], f16, kind="ExternalInput")
    xeo = nc.dram_tensor("xeo", [B, FEXT, cfg.tiles * P], f16,
                         kind="ExternalInput")
    w1e = nc.dram_tensor("w1e", [FEXT, H], f16, kind="ExternalInput")
    idxt = nc.dram_tensor("idxt", [P, cfg.idxcols], i16, kind="ExternalInput")
    dlt = nc.dram_tensor("dlt", [P, cfg.nchunks_total], f16, kind="ExternalInput")
    dcq = nc.dram_tensor("dcq", [P, cfg.tiles], f32, kind="ExternalInput")
    iot = nc.dram_tensor("iot", [P, P], f16, kind="ExternalInput")
    if has_b1:
        disc = nc.dram_tensor("disc", [P, cfg.tiles], f32, kind="ExternalInput")
        cct = nc.dram_tensor("cct", [P, cfg.tiles], f32, kind="ExternalInput")
        b1b = nc.dram_tensor("b1b", [P, YW], f32, kind="ExternalInput")
    y = nc.dram_tensor("y", [cfg.npad, YW], f16, kind="Internal")
    accd = nc.dram_tensor("acc", [P, YW], f32, kind="ExternalOutput")

    nblk = cfg.npad // 512
    rotpre = [[sum(cfg.rot[i][s] for i in range(r)) for s in range(cfg.nsc)]
              for r in range(cfg.group)]

    with tile.TileContext(nc) as tc:
        nc.gpsimd.load_library(mlp)
        with (
            tc.tile_pool(name="const", bufs=1) as cpool,
            tc.tile_pool(name="ph1", bufs=4) as p1pool,
            tc.tile_pool(name="ysb", bufs=3) as ypool,
            tc.tile_pool(name="gat", bufs=4) as gpool,
            tc.tile_pool(name="oh", bufs=8) as ohpool,
            tc.tile_pool(name="x1c", bufs=4) as xpool,
            tc.tile_pool(name="psy", bufs=2, space="PSUM") as psy,
            tc.tile_pool(name="psa", bufs=6, space="PSUM") as psa,
            ExitStack() as ctx,
        ):
            # constants / small preloads
            w1_sb = cpool.tile([FEXT, H], f16, tag="w1")
            nc.sync.dma_start(w1_sb[:], w1e[:])
            iota_sb = cpool.tile([P, P], f16, tag="iota")
            nc.sync.dma_start(iota_sb[:], iot[:])
            dl_sb = cpool.tile([P, cfg.nchunks_total], f16, tag="dl")
            nc.sync.dma_start(dl_sb[:], dlt[:])
            dcq_sb = cpool.tile([P, cfg.tiles], f32, tag="dcq")
            nc.sync.dma_start(dcq_sb[:], dcq[:])
            if has_b1:
                disc_sb = cpool.tile([P, cfg.tiles], f32, tag="disc")
                nc.sync.dma_start(disc_sb[:], disc[:])
                cc_sb = cpool.tile([P, cfg.tiles], f32, tag="cc")
                nc.sync.dma_start(cc_sb[:], cct[:])
                b1_sb = cpool.tile([P, YW], f32, tag="b1b")
                nc.sync.dma_start(b1_sb[:], b1b[:])
            acc_sb = cpool.tile([P, YW], f32, tag="acc")
            nc.vector.memset(acc_sb[:], 0)

            # ---- Phase 1: y[node] = dis*(x @ W1ext), fp16 rows [node, 2*H]
            for blk in range(nblk):
                n0 = blk * 512
                xts = []
                for b in range(B):
                    xt = p1pool.tile([FEXT, 512], f16, tag=f"xt{b}")
                    nc.sync.dma_start(xt[:], xe[b, :, n0:n0 + 512])
                    xts.append(xt)
                for sub in range(4):
                    ysb = ypool.tile([P, YW], f16, tag="ysb")
                    for b in range(B):
                        ps = psy.tile([P, H], f32, tag="psy")
                        nc.tensor.matmul(
                            ps[:],
                            lhsT=xts[b][:, sub * P:(sub + 1) * P],
                            rhs=w1_sb[:], start=True, stop=True)
                        nc.scalar.activation(
                            out=ysb[:, b * H:(b + 1) * H], in_=ps[:],
                            func=mybir.ActivationFunctionType.Copy)
                    r0 = n0 + sub * P
                    nc.sync.dma_start(y[r0:r0 + P, :], ysb[:])

            # ---- Phase 2: gather + segmented one-hot matmul + accumulate
            for g in range(cfg.ngroups):
                pst = [psa.tile([P, YW], f32, tag="psa", name=f"pst{g}_{i}")
                       for i in range(cfg.group)]
                # self-loop inputs: dis^2-scaled own features (see xeo build)
                xos = []
                for b in range(B):
                    xo = p1pool.tile([FEXT, cfg.group * P], f16, tag=f"xo{b}")
                    nc.sync.dma_start(
                        xo[:], xeo[b, :, g * cfg.group * P:(g + 1) * cfg.group * P])
                    xos.append(xo)
                start_mm = [None] * cfg.group
                for s in range(cfg.nsc):
                    call = g * cfg.nsc + s
                    ic0 = call * (cfg.call_idx // 16)
                    idx_sb = gpool.tile([P, cfg.call_idx // 16], i16, tag="idx")
                    nc.sync.dma_start(
                        idx_sb[:], idxt[:, ic0:ic0 + cfg.call_idx // 16])
                    gt = gpool.tile([P, cfg.call_chunks, YW], f16, tag="gt")
                    r0 = s * cfg.srcchunk
                    nc.gpsimd.dma_gather(
                        gt[:], y[r0:r0 + cfg.srcchunk, :], idx_sb[:],
                        cfg.call_idx, cfg.call_idx, YW, queue_num=s)
                    for ti in range(cfg.group):
                        t = g * cfg.group + ti
                        k = cfg.rot[ti][s]
                        off = rotpre[ti][s]
                        for j in range(k):
                            # global chunk column for dstloc:
                            gcol = call * cfg.call_chunks + off + j
                            oh = ohpool.tile([P, P], f16, tag="oh")
                            nc.vector.tensor_tensor(
                                out=oh[:],
                                in0=dl_sb[:, gcol:gcol + 1].to_broadcast([P, P]),
                                in1=iota_sb[:],
                                op=mybir.AluOpType.is_equal)
                            # exactly one start=True matmul per psum tile (PSUM
                            # zero-regions are 2KB-wide: start marks the whole
                            # region pending-zero, so it must be unique + first)
                            is_start = (s == 0 and j == 0)
                            mm = nc.tensor.matmul(
                                pst[ti][:], lhsT=oh[:],
                                rhs=gt[:, off + j, :],
                                start=is_start,
                                stop=(s == cfg.nsc - 1 and j == k - 1))
                            if is_start:
                                start_mm[ti] = mm
                                # self-loop term: accumulate xeo @ W1ext into
                                # each batch half, after the start matmul
                                for b in range(B):
                                    sm = nc.tensor.matmul(
                                        pst[ti][:, b * H:(b + 1) * H],
                                        lhsT=xos[b][:, ti * P:(ti + 1) * P],
                                        rhs=w1_sb[:], start=False, stop=False)
                                    bass._add_dep_helper(
                                        sm.ins, start_mm[ti].ins, sync=False,
                                        reason="self-mm after psum start")
                            else:
                                bass._add_dep_helper(
                                    mm.ins, start_mm[ti].ins, sync=False,
                                    reason="accum after psum start")
                for ti in range(cfg.group):
                    t = g * cfg.group + ti
                    x1c = xpool.tile([P, YW], f32, tag="x1c")
                    if not has_b1:
                        # x1c = relu(psum * (dis*c))   (valid since c>0)
                        nc.scalar.activation(
                            out=x1c[:], in_=pst[ti][:],
                            func=mybir.ActivationFunctionType.Relu,
                            bias=0.0, scale=dcq_sb[:, t:t + 1])
                    else:
                        t1 = xpool.tile([P, YW], f32, tag="t1")
                        nc.vector.tensor_scalar(
                            out=t1[:], in0=pst[ti][:],
                            scalar1=disc_sb[:, t:t + 1], scalar2=None,
                            op0=mybir.AluOpType.mult)
                        nc.vector.tensor_tensor(
                            out=t1[:], in0=t1[:], in1=b1_sb[:],
                            op=mybir.AluOpType.add)
                        nc.scalar.activation(
                            out=t1[:], in_=t1[:],
                            func=mybir.ActivationFunctionType.Relu)
                        nc.vector.tensor_scalar(
                            out=x1c[:], in0=t1[:],
                            scalar1=cc_sb[:, t:t + 1], scalar2=None,
                            op0=mybir.AluOpType.mult)
                    nc.vector.tensor_tensor(
                        out=acc_sb[:], in0=acc_sb[:], in1=x1c[:],
                        op=mybir.AluOpType.add)

            nc.sync.dma_start(accd[:], acc_sb[:])

    nc.compile()
    return nc


_PROG_CACHE = {}


def _get_program(cfg: Cfg, has_b1: bool):
    key = (cfg, has_b1)
    if key not in _PROG_CACHE:
        _PROG_CACHE[key] = _build_program(cfg, has_b1)
    return _PROG_CACHE[key]


def _pack_core(cfg: Cfg, core, src, dst, dis_c, n_nodes):
    """Bin-pack this core's dst nodes into tiles; build gather/dstloc/dcq data.

    Returns (idx_w [128, idxcols] i16, dl_w [128, nchunks] f16,
             dcq_w [128, tiles] f32, tile_of, slot_of)."""
    n0 = core * cfg.ndst
    sel = (dst >= n0) & (dst < n0 + cfg.ndst)
    es = src[sel]
    ed = dst[sel]
    # (self edges are handled by the xeown direct matmul, not the gather)
    dl = ed - n0                       # local dst id
    sc = es // cfg.srcchunk            # src chunk of each edge

    cnt = np.bincount(dl * cfg.nsc + sc, minlength=cfg.ndst * cfg.nsc)
    cnt = cnt.reshape(cfg.ndst, cfg.nsc)

    rot = np.array(cfg.rot, dtype=np.int64)          # [4, nsc]
    caps = (rot[np.arange(cfg.tiles) % cfg.nsc] * P).copy()  # [tiles, nsc]
    for s in range(cfg.nsc):
        assert cnt[:, s].sum() <= caps[:, s].sum(), \
            f"core {core}: src chunk {s} demand exceeds capacity"

    order = np.argsort(-cnt.sum(1), kind="stable")
    slots_used = np.zeros(cfg.tiles, dtype=np.int64)
    tile_of = np.full(cfg.ndst, -1, dtype=np.int64)
    slot_of = np.full(cfg.ndst, -1, dtype=np.int64)
    for nloc in order:
        need = cnt[nloc]
        ok = (caps >= need).all(axis=1) & (slots_used < P)
        if not ok.any():
            raise RuntimeError(f"core {core}: bin packing failed for node {nloc}")
        # best fit = feasible tile with most remaining capacity (balances load;
        # with exact slot counts every tile must end up full)
        score = caps.sum(axis=1) + (P - slots_used)
        score[~ok] = -1
        t = int(np.argmax(score))
        tile_of[nloc] = t
        slot_of[nloc] = slots_used[t]
        slots_used[t] += 1
        caps[t] -= need

    # edge stream positions
    et = tile_of[dl]
    eslot = slot_of[dl]
    o = np.lexsort((sc, et))
    et_s, sc_s, slot_s, src_s = et[o], sc[o], eslot[o], es[o]
    ks = et_s * cfg.nsc + sc_s
    counts = np.bincount(ks, minlength=cfg.tiles * cfg.nsc)
    gbase = np.concatenate([[0], np.cumsum(counts)[:-1]])
    rank = np.arange(len(ks)) - gbase[ks]

    # padded stream base for (t, s)
    rotpre = np.zeros((cfg.nsc, cfg.nsc), dtype=np.int64)  # [r, s] prefix
    for r in range(cfg.nsc):
        for s in range(cfg.nsc):
            rotpre[r, s] = sum(cfg.rot[i][s] for i in range(r))
    tt = np.arange(cfg.tiles)
    callno = (tt // cfg.group)[:, None] * cfg.nsc + np.arange(cfg.nsc)[None, :]
    pbase = callno * cfg.call_idx + rotpre[tt % cfg.group] * P  # [tiles, nsc]
    assert (counts.reshape(cfg.tiles, cfg.nsc) <= rot[tt % cfg.nsc] * P).all()

    total = cfg.ncalls * cfg.call_idx
    idx_flat = np.zeros(total, dtype=np.int16)
    dl_flat = np.full(total, 255.0, dtype=np.float16)
    pos = pbase[et_s, sc_s] + rank
    idx_flat[pos] = (src_s - sc_s * cfg.srcchunk).astype(np.int16)
    dl_flat[pos] = slot_s.astype(np.float16)

    ci = cfg.call_idx
    idx_w = idx_flat.reshape(cfg.ncalls, ci // 16, 16).transpose(2, 0, 1)
    idx_w = np.tile(idx_w.reshape(16, -1), (8, 1))           # [128, idxcols]
    dl_w = dl_flat.reshape(cfg.nchunks_total, P).T.copy()    # [128, nchunks]

    dcq_w = np.zeros((P, cfg.tiles), dtype=np.float32)
    dcq_w[slot_of, tile_of] = dis_c[n0:n0 + cfg.ndst]
    return idx_w, dl_w, dcq_w, tile_of, slot_of


def _prepare(cfg: Cfg, node, node_type, edge_index, embed, W1, b1, W2, b2):
    n = cfg.n
    src = edge_index[0].astype(np.int64)
    dst = edge_index[1].astype(np.int64)
    deg = (np.bincount(dst, minlength=n) + 1).astype(np.float32)
    dis = (1.0 / np.sqrt(deg.astype(np.float64))).astype(np.float32)
    s_arr = np.bincount(src, weights=dis[dst].astype(np.float64), minlength=n)
    c = (dis.astype(np.float64) * (s_arr + dis)).astype(np.float32)
    dis_c = (dis.astype(np.float64) * c).astype(np.float32)

    T8 = (embed.astype(np.float64) @ W1[F_IN:, :].astype(np.float64))
    w1e = np.concatenate([W1[:F_IN, :], T8.astype(np.float32)], axis=0)
    w1e = np.ascontiguousarray(w1e, dtype=np.float16)

    xe = np.zeros((B, FEXT, cfg.npad), dtype=np.float16)
    xe[:, :F_IN, :n] = (node.transpose(0, 2, 1)
                        * dis[None, None, :]).astype(np.float16)
    oh8 = np.zeros((8, n), dtype=np.float32)
    oh8[node_type.astype(np.int64), np.arange(n)] = dis
    xe[:, F_IN:, :n] = oh8[None].astype(np.float16)

    iota = np.tile(np.arange(P, dtype=np.float16), (P, 1))

    has_b1 = bool(np.any(b1 != 0))
    in_maps = []
    metas = []
    for core in range(cfg.ncores):
        idx_w, dl_w, dcq_w, tile_of, slot_of = _pack_core(
            cfg, core, src, dst, dis_c, n)
        # xeown: own nodes' features at (tile, slot) columns. xe already
        # carries one dis factor, so xeown @ W1ext = dis*xw = y[n], exactly the
        # self-loop row the segment sum needs (psum is scaled by dis*c later).
        n0 = core * cfg.ndst
        perm = np.full(cfg.tiles * P, -1, dtype=np.int64)
        perm[tile_of * P + slot_of] = np.arange(n0, n0 + cfg.ndst)
        used = perm >= 0
        xeo = np.zeros((B, FEXT, cfg.tiles * P), dtype=np.float16)
        xeo[:, :, used] = xe[:, :, perm[used]]
        m = {"xe": xe, "xeo": xeo, "w1e": w1e, "idxt": idx_w, "dlt": dl_w,
             "dcq": dcq_w, "iot": iota}
        if has_b1:
            disc_w = np.zeros((P, cfg.tiles), dtype=np.float32)
            cc_w = np.zeros((P, cfg.tiles), dtype=np.float32)
            n0 = core * cfg.ndst
            disc_w[slot_of, tile_of] = dis[n0:n0 + cfg.ndst]
            cc_w[slot_of, tile_of] = c[n0:n0 + cfg.ndst]
            m["disc"] = disc_w
            m["cct"] = cc_w
            m["b1b"] = np.tile(b1.astype(np.float32), (P, B))
        in_maps.append(m)
        metas.append((tile_of, slot_of))
    return in_maps, has_b1


def run(inputs, cfg: Cfg = CFG, trace: bool = False):
    node = np.asarray(inputs["node"], dtype=np.float32)
    node_type = np.asarray(inputs["node_type"])
    edge_index = np.asarray(inputs["edge_index"])
    embed = np.asarray(inputs["embed"], dtype=np.float32)
    W1 = np.asarray(inputs["W1"], dtype=np.float32)
    b1 = np.asarray(inputs["b1"], dtype=np.float32)
    W2 = np.asarray(inputs["W2"], dtype=np.float32)
    b2 = np.asarray(inputs["b2"], dtype=np.float32)

    in_maps, has_b1 = _prepare(cfg, node, node_type, edge_index,
                               embed, W1, b1, W2, b2)
    nc = _get_program(cfg, has_b1)
    res = run_bass_kernel_spmd(
        nc, in_maps, core_ids=list(range(cfg.ncores)), trace=trace,
        trace_cores=list(range(cfg.ncores)) if trace else None)

    total = np.zeros((B, H), dtype=np.float64)
    for core in range(cfg.ncores):
        acc = res.results[core]["acc"].astype(np.float64)   # [128, 2*H]
        total += acc.reshape(P, B, H).sum(axis=0)
    out = (total @ W2.astype(np.float64)) / cfg.n + b2.astype(np.float64)
    return out.astype(np.float32), res


def kernel(**inputs) -> np.ndarray:
    out, _ = run(inputs, CFG, trace=False)
    return out



# revision 9
# speedup vs baseline: 6.5124x; 1.1990x over previous
"""Trainium2 Bass kernel for nn_Encoder — v2 "aggregate-first".

Math (exact, up to float reordering):
  S[n] = (sum_{e: dst=n} xg[src_e]) + xg[n],   xg[m] = dis_m * xext_m
  x1[n] = relu(dis_n * c_n * (S[n] @ W1ext))   (b1 == 0 path; c,dis > 0)
  out = (1/N) * (sum_n x1[n]) @ W2 + b2        (layer 2 + mean collapsed)

v2 removes the baseline's dense phase 1 (y = x@W1 table) entirely: edges
gather raw (dis-scaled) feature rows xg directly from DRAM, self-loops are
ordinary edges in the stream, aggregation happens in 124-dim feature space
via one-hot matmuls into PSUM, and W1 is applied per dst tile after a
PE-transpose. Gathers run on all 4 SWDGE queues (one per src chunk) so the
4 Q7 pairs generate descriptors concurrently.

Sharding: dst nodes (and their incoming edges) split across 8 cores; xg
table replicated; per-core [128, 2*H] partial accumulations summed on host.
"""

import sys, os, types
sys.path.insert(0, "/opt/trn_rl_repo")

# antenv.axon_hooks shim (image's antenv stub lacks it); needed for NTFF trace.
if "antenv.axon_hooks" not in sys.modules:
    _hook = [None]
    _m = types.ModuleType("antenv.axon_hooks")
    _m.set_axon_ntff_profile_hook = lambda h: _hook.__setitem__(0, h)
    _m.get_axon_ntff_profile_hook = lambda: _hook[0]
    sys.modules["antenv.axon_hooks"] = _m
    try:
        import antenv
        antenv.axon_hooks = _m
        from trn_agent_boot.trn_boot import _ntff_profile_via_ctypes
        _m.set_axon_ntff_profile_hook(
            _ntff_profile_via_ctypes("/opt/axon/libaxon_pjrt.so"))
    except Exception:
        pass

import numpy as np
from contextlib import ExitStack
from dataclasses import dataclass

import concourse.bacc as bacc
import concourse.bass as bass
import concourse.mybir as mybir
import concourse.tile as tile
from concourse.bass_utils import run_bass_kernel_spmd
from concourse.library_config import mlp

P = 128
H = 128
F_IN = 116
FEXT = F_IN + 8          # 124 features (node + one-hot type), padded to 128
FPAD = 128
B = 2
XW = B * FPAD            # 256: xg row elements (both batches, padded)
YW = B * H               # 256: output row elements


@dataclass(frozen=True)
class Cfg:
    n: int = 100000      # nodes
    ncores: int = 8
    tiles: int = 104     # dst tiles per core (128 slots each)
    chunks: int = 16     # 128-edge chunks per tile (sum over 4 src chunks)
    group: int = 2       # tiles per gather-call group (= psum tiles in flight)
    nsc: int = 4         # src chunks (int16 gather index reach)

    @property
    def ndst(self):
        return self.n // self.ncores

    @property
    def srcchunk(self):
        return -(-self.n // self.nsc)

    @property
    def rot(self):        # rot[r][s]: chunks of tile (t%group==r) in src chunk s
        base, extra = divmod(self.chunks, self.nsc)
        return [[base + (1 if (s - r) % self.nsc < extra else 0)
                 for s in range(self.nsc)] for r in range(self.nsc)]

    @property
    def ngroups(self):
        return self.tiles // self.group

    @property
    def call_chunks(self):
        return sum(self.rot[r][0] for r in range(self.group))

    @property
    def call_idx(self):
        return self.call_chunks * P

    @property
    def ncalls(self):
        return self.ngroups * self.nsc

    @property
    def idxcols(self):
        return self.ncalls * (self.call_idx // 16)

    @property
    def nchunks_total(self):
        return self.tiles * self.chunks


CFG = Cfg()

f32 = mybir.dt.float32
f16 = mybir.dt.float16
i16 = mybir.dt.int16


def _build_program(cfg: Cfg, has_b1: bool):
    nc = bacc.Bacc("TRN2", num_swdge_queues=4)
    npad2 = cfg.nsc * cfg.srcchunk
    xg = nc.dram_tensor("xg", [npad2, XW], f16, kind="ExternalInput")
    xgo = nc.dram_tensor("xgo", [cfg.tiles * P, XW], f16, kind="ExternalInput")
    w1e = nc.dram_tensor("w1e", [FPAD, H], f16, kind="ExternalInput")
    idxt = nc.dram_tensor("idxt", [P, cfg.idxcols], i16, kind="ExternalInput")
    dlt = nc.dram_tensor("dlt", [P, cfg.nchunks_total], f16, kind="ExternalInput")
    dcq = nc.dram_tensor("dcq", [P, cfg.tiles], f32, kind="ExternalInput")
    iot = nc.dram_tensor("iot", [P, P], f16, kind="ExternalInput")
    idn = nc.dram_tensor("idn", [P, P], f16, kind="ExternalInput")
    if has_b1:
        disc = nc.dram_tensor("disc", [P, cfg.tiles], f32, kind="ExternalInput")
        cct = nc.dram_tensor("cct", [P, cfg.tiles], f32, kind="ExternalInput")
        b1b = nc.dram_tensor("b1b", [P, YW], f32, kind="ExternalInput")
    accd = nc.dram_tensor("acc", [P, YW], f32, kind="ExternalOutput")

    rotpre = [[sum(cfg.rot[i][s] for i in range(r)) for s in range(cfg.nsc)]
              for r in range(cfg.group)]

    with tile.TileContext(nc) as tc:
        nc.gpsimd.load_library(mlp)
        with (
            tc.tile_pool(name="const", bufs=1) as cpool,
            tc.tile_pool(name="gat", bufs=10) as gpool,
            tc.tile_pool(name="xop", bufs=4) as xopool,
            tc.tile_pool(name="oh", bufs=10) as ohpool,
            tc.tile_pool(name="agg", bufs=4) as apool,
            tc.tile_pool(name="x1c", bufs=4) as xpool,
            tc.tile_pool(name="psag", bufs=4, space="PSUM") as psag,
            tc.tile_pool(name="pstr", bufs=1, space="PSUM") as pstr,
            tc.tile_pool(name="ps2", bufs=2, space="PSUM") as ps2,
            # psag 4 + pstr 1 + ps2 2 <= 8 PSUM banks
            ExitStack() as ctx,
        ):
            # constants / small preloads
            w1_sb = cpool.tile([FPAD, H], f16, tag="w1")
            nc.sync.dma_start(w1_sb[:], w1e[:])
            iota_sb = cpool.tile([P, P], f16, tag="iota")
            nc.sync.dma_start(iota_sb[:], iot[:])
            iden_sb = cpool.tile([P, P], f16, tag="iden")
            nc.sync.dma_start(iden_sb[:], idn[:])
            dl_sb = cpool.tile([P, cfg.nchunks_total], f16, tag="dl")
            nc.sync.dma_start(dl_sb[:], dlt[:])
            dcq_sb = cpool.tile([P, cfg.tiles], f32, tag="dcq")
            nc.sync.dma_start(dcq_sb[:], dcq[:])
            if has_b1:
                disc_sb = cpool.tile([P, cfg.tiles], f32, tag="disc")
                nc.sync.dma_start(disc_sb[:], disc[:])
                cc_sb = cpool.tile([P, cfg.tiles], f32, tag="cc")
                nc.sync.dma_start(cc_sb[:], cct[:])
                b1_sb = cpool.tile([P, YW], f32, tag="b1b")
                nc.sync.dma_start(b1_sb[:], b1b[:])
            acc_sb = cpool.tile([P, YW], f32, tag="acc")
            nc.vector.memset(acc_sb[:], 0)

            ic_g = cfg.nsc * (cfg.call_idx // 16)  # idx cols per group
            for g in range(cfg.ngroups):
                pst = [psag.tile([P, XW], f32, tag="psag", name=f"pst{g}_{i}")
                       for i in range(cfg.group)]
                # one idx DMA for the whole group's 4 gather calls
                idx_sb = gpool.tile([P, ic_g], i16, tag="idx")
                nc.sync.dma_start(
                    idx_sb[:], idxt[:, g * ic_g:(g + 1) * ic_g])
                # self-loop rows double as the psum-start matmul:
                # pst[ti] = I^T @ xgo_tile  (+= one-hot aggregation after)
                start_mm = [None] * cfg.group
                for ti in range(cfg.group):
                    t = g * cfg.group + ti
                    xo = xopool.tile([P, XW], f16, tag=f"xo{ti}")
                    nc.sync.dma_start(xo[:], xgo[t * P:(t + 1) * P, :])
                    start_mm[ti] = nc.tensor.matmul(
                        pst[ti][:], lhsT=iden_sb[:], rhs=xo[:],
                        start=True, stop=False)
                for s in range(cfg.nsc):
                    call = g * cfg.nsc + s
                    gt = gpool.tile([P, cfg.call_chunks, XW], f16, tag="gt")
                    r0 = s * cfg.srcchunk
                    nc.gpsimd.dma_gather(
                        gt[:], xg[r0:r0 + cfg.srcchunk, :],
                        idx_sb[:, s * (cfg.call_idx // 16):
                               (s + 1) * (cfg.call_idx // 16)],
                        cfg.call_idx, cfg.call_idx, XW, queue_num=s)
                    for ti in range(cfg.group):
                        k = cfg.rot[ti][s]
                        off = rotpre[ti][s]
                        for j in range(k):
                            gcol = call * cfg.call_chunks + off + j
                            oh = ohpool.tile([P, P], f16, tag="oh")
                            nc.vector.tensor_tensor(
                                out=oh[:],
                                in0=dl_sb[:, gcol:gcol + 1].to_broadcast([P, P]),
                                in1=iota_sb[:],
                                op=mybir.AluOpType.is_equal)
                            mm = nc.tensor.matmul(
                                pst[ti][:], lhsT=oh[:],
                                rhs=gt[:, off + j, :],
                                start=False,
                                stop=(s == cfg.nsc - 1 and j == k - 1))
                            bass._add_dep_helper(
                                mm.ins, start_mm[ti].ins, sync=False,
                                reason="accum after psum start")
                for ti in range(cfg.group):
                    t = g * cfg.group + ti
                    # agg [slot, 2*FPAD] f32 -> f16 in SBUF
                    aggS = apool.tile([P, XW], f16, tag="aggS")
                    nc.scalar.activation(
                        out=aggS[:], in_=pst[ti][:],
                        func=mybir.ActivationFunctionType.Copy)
                    # transpose each batch half: psumT_b [feat, slot] f16.
                    # Separate PSUM tiles per half: each transpose's start
                    # zero-fills its whole 2KB bank, so halves of one tile
                    # would wipe each other.
                    aggT = apool.tile([P, XW], f16, tag="aggT")
                    for b in range(B):
                        psT = pstr.tile([P, P], f16, tag=f"psT{b}",
                                        name=f"psT{g}_{ti}_{b}")
                        nc.tensor.transpose(
                            psT[:],
                            aggS[:, b * FPAD:(b + 1) * FPAD], iden_sb[:])
                        nc.scalar.activation(
                            out=aggT[:, b * FPAD:(b + 1) * FPAD], in_=psT[:],
                            func=mybir.ActivationFunctionType.Copy)
                    # W1 application: psum2[slot, b*H:(b+1)*H] = aggT_b^T @ W1.
                    # Chained start/stop: the first mm's start zero-fills the
                    # whole bank, second mm accumulates into its (zeroed) half.
                    psum2 = ps2.tile([P, YW], f32, tag="ps2")
                    mm0 = nc.tensor.matmul(
                        psum2[:, 0:H], lhsT=aggT[:, 0:FPAD],
                        rhs=w1_sb[:], start=True, stop=False)
                    mm1 = nc.tensor.matmul(
                        psum2[:, H:2 * H], lhsT=aggT[:, FPAD:2 * FPAD],
                        rhs=w1_sb[:], start=False, stop=True)
                    bass._add_dep_helper(
                        mm1.ins, mm0.ins, sync=False,
                        reason="second half after psum2 start")
                    x1c = xpool.tile([P, YW], f32, tag="x1c")
                    if not has_b1:
                        nc.scalar.activation(
                            out=x1c[:], in_=psum2[:],
                            func=mybir.ActivationFunctionType.Relu,
                            bias=0.0, scale=dcq_sb[:, t:t + 1])
                    else:
                        t1 = xpool.tile([P, YW], f32, tag="t1")
                        nc.vector.tensor_scalar(
                            out=t1[:], in0=psum2[:],
                            scalar1=disc_sb[:, t:t + 1], scalar2=None,
                            op0=mybir.AluOpType.mult)
                        nc.vector.tensor_tensor(
                            out=t1[:], in0=t1[:], in1=b1_sb[:],
                            op=mybir.AluOpType.add)
                        nc.scalar.activation(
                            out=t1[:], in_=t1[:],
                            func=mybir.ActivationFunctionType.Relu)
                        nc.vector.tensor_scalar(
                            out=x1c[:], in0=t1[:],
                            scalar1=cc_sb[:, t:t + 1], scalar2=None,
                            op0=mybir.AluOpType.mult)
                    nc.vector.tensor_tensor(
                        out=acc_sb[:], in0=acc_sb[:], in1=x1c[:],
                        op=mybir.AluOpType.add)

            nc.sync.dma_start(accd[:], acc_sb[:])

    nc.compile()
    return nc


_PROG_CACHE = {}


def _get_program(cfg: Cfg, has_b1: bool):
    key = (cfg, has_b1)
    if key not in _PROG_CACHE:
        _PROG_CACHE[key] = _build_program(cfg, has_b1)
    return _PROG_CACHE[key]


def _pack_core(cfg: Cfg, core, src, dst, dis_c, n_nodes):
    """Bin-pack this core's dst nodes into tiles; build gather/dstloc/dcq data.

    (self edges are handled by the xgo identity matmul, not the gather)
    Returns (idx_w [128, idxcols] i16, dl_w [128, nchunks] f16,
             dcq_w [128, tiles] f32, tile_of, slot_of)."""
    n0 = core * cfg.ndst
    sel = (dst >= n0) & (dst < n0 + cfg.ndst)
    es = src[sel]
    ed = dst[sel]
    dl = ed - n0                       # local dst id
    sc = es // cfg.srcchunk            # src chunk of each edge

    cnt = np.bincount(dl * cfg.nsc + sc, minlength=cfg.ndst * cfg.nsc)
    cnt = cnt.reshape(cfg.ndst, cfg.nsc)

    rot = np.array(cfg.rot, dtype=np.int64)          # [group, nsc]
    caps = (rot[np.arange(cfg.tiles) % cfg.group] * P).copy()  # [tiles, nsc]
    for s in range(cfg.nsc):
        assert cnt[:, s].sum() <= caps[:, s].sum(), \
            f"core {core}: src chunk {s} demand exceeds capacity"

    order = np.argsort(-cnt.sum(1), kind="stable")
    slots_used = np.zeros(cfg.tiles, dtype=np.int64)
    tile_of = np.full(cfg.ndst, -1, dtype=np.int64)
    slot_of = np.full(cfg.ndst, -1, dtype=np.int64)
    for nloc in order:
        need = cnt[nloc]
        ok = (caps >= need).all(axis=1) & (slots_used < P)
        if not ok.any():
            raise RuntimeError(f"core {core}: bin packing failed for node {nloc}")
        score = caps.sum(axis=1) + (P - slots_used)
        score[~ok] = -1
        t = int(np.argmax(score))
        tile_of[nloc] = t
        slot_of[nloc] = slots_used[t]
        slots_used[t] += 1
        caps[t] -= need

    # edge stream positions
    et = tile_of[dl]
    eslot = slot_of[dl]
    o = np.lexsort((sc, et))
    et_s, sc_s, slot_s, src_s = et[o], sc[o], eslot[o], es[o]
    ks = et_s * cfg.nsc + sc_s
    counts = np.bincount(ks, minlength=cfg.tiles * cfg.nsc)
    gbase = np.concatenate([[0], np.cumsum(counts)[:-1]])
    rank = np.arange(len(ks)) - gbase[ks]

    rotpre = np.zeros((cfg.group, cfg.nsc), dtype=np.int64)
    for r in range(cfg.group):
        for s in range(cfg.nsc):
            rotpre[r, s] = sum(cfg.rot[i][s] for i in range(r))
    tt = np.arange(cfg.tiles)
    callno = (tt // cfg.group)[:, None] * cfg.nsc + np.arange(cfg.nsc)[None, :]
    pbase = callno * cfg.call_idx + rotpre[tt % cfg.group] * P  # [tiles, nsc]
    assert (counts.reshape(cfg.tiles, cfg.nsc) <= rot[tt % cfg.group] * P).all()

    total = cfg.ncalls * cfg.call_idx
    idx_flat = np.zeros(total, dtype=np.int16)
    dl_flat = np.full(total, 255.0, dtype=np.float16)
    pos = pbase[et_s, sc_s] + rank
    idx_flat[pos] = (src_s - sc_s * cfg.srcchunk).astype(np.int16)
    dl_flat[pos] = slot_s.astype(np.float16)

    ci = cfg.call_idx
    idx_w = idx_flat.reshape(cfg.ncalls, ci // 16, 16).transpose(2, 0, 1)
    idx_w = np.tile(idx_w.reshape(16, -1), (8, 1))           # [128, idxcols]
    dl_w = dl_flat.reshape(cfg.nchunks_total, P).T.copy()    # [128, nchunks]

    dcq_w = np.zeros((P, cfg.tiles), dtype=np.float32)
    dcq_w[slot_of, tile_of] = dis_c[n0:n0 + cfg.ndst]
    return idx_w, dl_w, dcq_w, tile_of, slot_of


def _prepare(cfg: Cfg, node, node_type, edge_index, embed, W1, b1, W2, b2):
    n = cfg.n
    src = edge_index[0].astype(np.int64)
    dst = edge_index[1].astype(np.int64)
    deg = (np.bincount(dst, minlength=n) + 1).astype(np.float32)
    dis = (1.0 / np.sqrt(deg.astype(np.float64))).astype(np.float32)
    s_arr = np.bincount(src, weights=dis[dst].astype(np.float64), minlength=n)
    c = (dis.astype(np.float64) * (s_arr + dis)).astype(np.float32)
    dis_c = (dis.astype(np.float64) * c).astype(np.float32)

    T8 = (embed.astype(np.float64) @ W1[F_IN:, :].astype(np.float64))
    w1e = np.zeros((FPAD, H), dtype=np.float16)
    w1e[:F_IN] = W1[:F_IN, :].astype(np.float16)
    w1e[F_IN:FEXT] = T8.astype(np.float16)

    npad2 = cfg.nsc * cfg.srcchunk
    xgf = np.zeros((npad2, B, FPAD), dtype=np.float16)
    xgf[:n, :, :F_IN] = (node.transpose(1, 0, 2)
                         * dis[:, None, None]).astype(np.float16)
    oh8 = np.zeros((n, 8), dtype=np.float32)
    oh8[np.arange(n), node_type.astype(np.int64)] = dis
    xgf[:n, :, F_IN:FEXT] = oh8[:, None, :].astype(np.float16)
    xg = xgf.reshape(npad2, B * FPAD)

    iota = np.tile(np.arange(P, dtype=np.float16), (P, 1))
    iden = np.eye(P, dtype=np.float16)

    has_b1 = bool(np.any(b1 != 0))
    in_maps = []
    for core in range(cfg.ncores):
        idx_w, dl_w, dcq_w, tile_of, slot_of = _pack_core(
            cfg, core, src, dst, dis_c, n)
        # xgo: own nodes' xg rows at (tile, slot) positions -> the self-loop
        # term enters psag via one identity matmul per tile.
        n0 = core * cfg.ndst
        xgo_w = np.zeros((cfg.tiles * P, XW), dtype=np.float16)
        xgo_w[tile_of * P + slot_of] = xg[n0:n0 + cfg.ndst]
        m = {"xg": xg, "xgo": xgo_w, "w1e": w1e, "idxt": idx_w, "dlt": dl_w,
             "dcq": dcq_w, "iot": iota, "idn": iden}
        if has_b1:
            disc_w = np.zeros((P, cfg.tiles), dtype=np.float32)
            cc_w = np.zeros((P, cfg.tiles), dtype=np.float32)
            n0 = core * cfg.ndst
            disc_w[slot_of, tile_of] = dis[n0:n0 + cfg.ndst]
            cc_w[slot_of, tile_of] = c[n0:n0 + cfg.ndst]
            m["disc"] = disc_w
            m["cct"] = cc_w
            m["b1b"] = np.tile(b1.astype(np.float32), (P, B))
        in_maps.append(m)
    return in_maps, has_b1


def run(inputs, cfg: Cfg = CFG, trace: bool = False):
    node = np.asarray(inputs["node"], dtype=np.float32)
    node_type = np.asarray(inputs["node_type"])
    edge_index = np.asarray(inputs["edge_index"])
    embed = np.asarray(inputs["embed"], dtype=np.float32)
    W1 = np.asarray(inputs["W1"], dtype=np.float32)
    b1 = np.asarray(inputs["b1"], dtype=np.float32)
    W2 = np.asarray(inputs["W2"], dtype=np.float32)
    b2 = np.asarray(inputs["b2"], dtype=np.float32)

    in_maps, has_b1 = _prepare(cfg, node, node_type, edge_index,
                               embed, W1, b1, W2, b2)
    nc = _get_program(cfg, has_b1)
    res = run_bass_kernel_spmd(
        nc, in_maps, core_ids=list(range(cfg.ncores)), trace=trace,
        trace_cores=list(range(cfg.ncores)) if trace else None)

    total = np.zeros((B, H), dtype=np.float64)
    for core in range(cfg.ncores):
        acc = res.results[core]["acc"].astype(np.float64)   # [128, 2*H]
        total += acc.reshape(P, B, H).sum(axis=0)
    out = (total @ W2.astype(np.float64)) / cfg.n + b2.astype(np.float64)
    return out.astype(np.float32), res


def kernel(**inputs) -> np.ndarray:
    out, _ = run(inputs, CFG, trace=False)
    return out
